# revision 16
# baseline (speedup 1.0000x reference)
"""Causal self-attention (B=2, T=2048, C=1024, H=16, D=64) on 8 trn2 NeuronCores.

Sharding: core c -> batch b = c // 4, head group g = c % 4 (heads 4g..4g+3).
Each core computes, for its batch and its 4 heads:
    qkT   = Wqk_local^T @ x_b^T          [512, 2048]   (q/k transposed layout)
    v     = x_b @ Wv_local               [2048, 256]   (natural layout)
    sT    = k q^T (per head)             [k, q] blocks; exp(s/8), causal mask
    pv    = (v|ones)^T @ exp(sT)         [128, q]: 64 attn rows + 64 denom rows
    y_par = attnT-contraction @ Wp_local [2048, 1024]
Host: y[b] = sum of the 4 partials + b_proj + (b_attn_v @ W_proj).

The host pre-transposes x (layout choice only - all FLOPs stay on device)
and column/row-shards the weights. b_attn(q,k) folded in via per-partition
activation bias; b_attn(v) and b_proj folded in on the host (exact since
softmax rows sum to 1).

Engine-lane constraint: DVE/ACT operands must share the partition window, so
attention rows live at partitions 0:64 for even heads and 64:128 for odd
heads (the v|ones weight column order flips per parity), and the reciprocal
row block is mirrored across the partition halves with a tiny SBUF->SBUF DMA.
"""

import os
import sys

import numpy as np

for _p in ("/opt/trn_rl_repo",):
    if _p not in sys.path:
        sys.path.insert(0, _p)

import concourse.bass as bass  # noqa: E402,F401
import concourse.mybir as mybir  # noqa: E402
import concourse.tile as tile  # noqa: E402
from concourse import bacc  # noqa: E402
from concourse.bass_utils import run_bass_kernel_spmd  # noqa: E402

B, T, C, H, D = 2, 2048, 1024, 16, 64
HL = 4          # heads per core
N_CORES = 8
QCH = 512       # q-chunk width (one PSUM bank of fp32)
NKT = T // 128  # 16 k-tiles per head
NQC = T // QCH  # 4 q-chunks

F32 = mybir.dt.float32

# matmul compute dtype: "f32" (4 cyc/row) or "f32r" (1 cyc/row for moving
# dim >= 256; fp32 stored with mantissa rounded to 11 bits, ~1.2e-4 rel)
MM_DT = os.environ.get("KMM_DT", "f32r")
MMDT = {"f32": F32, "f32r": mybir.dt.float32r}[MM_DT]

LAST_RESULT = None  # BassKernelResults of the most recent kernel() call


def round_f32r(a):
    """Round-to-nearest-even fp32 -> fp32r (11-bit mantissa, low 12 bits 0)."""
    if MM_DT != "f32r":
        return a
    u = np.ascontiguousarray(a, np.float32).view(np.uint32)
    u = (u + 0x7FF + ((u >> 12) & 1)) & np.uint32(0xFFFFF000)
    return u.view(np.float32)


def _body(tc, debug_dumps=False):
    nc = tc.nc
    ACT = mybir.ActivationFunctionType

    xt = nc.dram_tensor("xt", [C, T], MMDT, kind="ExternalInput").ap()
    wqk = nc.dram_tensor("wqk", [C, 512], MMDT, kind="ExternalInput").ap()
    wv = nc.dram_tensor("wv", [C, 256], MMDT, kind="ExternalInput").ap()
    wp = nc.dram_tensor("wp", [256, C], MMDT, kind="ExternalInput").ap()
    bqk = nc.dram_tensor("bqk", [128, 4], F32, kind="ExternalInput").ap()
    tri = nc.dram_tensor("tri", [128, 128], MMDT, kind="ExternalInput").ap()
    trix = nc.dram_tensor("trix", [128, 256], MMDT, kind="ExternalInput").ap()
    y = nc.dram_tensor("y", [T, C], F32, kind="ExternalOutput").ap()

    # ---------------- persistent SBUF ----------------
    persist = tc.alloc_tile_pool(name="persist", bufs=1)
    qk_sb = persist.tile([128, 2, 2, T], MMDT, tag="qk")    # [p, hpair, q/k, t]
    v_sb = persist.tile([128, NKT, HL, 128], MMDT, tag="v")  # [p, ktile, h, d|ones]
    at_sb = persist.tile([128, 2, T], MMDT, tag="at")       # attnT [p, ctile, t]
    wp_sb = persist.tile([128, 2, C], MMDT, tag="wp")
    bqk_sb = persist.tile([128, 4], F32, tag="bqk")
    tri_sb = persist.tile([128, 128], MMDT, tag="tri")
    trix_sb = persist.tile([128, 256], MMDT, tag="trix")

    nc.sync.dma_start(out=wp_sb, in_=wp.rearrange("(c p) n -> p c n", p=128))
    nc.sync.dma_start(out=bqk_sb, in_=bqk)
    nc.sync.dma_start(out=tri_sb, in_=tri)
    nc.sync.dma_start(out=trix_sb, in_=trix)
    # ones|v weight layout (all heads): ones cols 0:64 -> denominator rows 0:64
    # of the PV psum; v cols 64:128 -> attn rows 64:128.  (reciprocal_approx
    # is a custom DVE op that only works at partition base 0, so the denom
    # must always land in the low half.)  memset can't write f32r, so the
    # ones come from tri's all-ones last column, free-broadcast by the DVE.
    nc.vector.tensor_copy(
        out=v_sb[:, :, :, 0:64],
        in_=tri_sb[:, 127:128].broadcast_to([128, NKT, HL, 64]))

    # ---------------- phases A-C: load x^T & W, qkv projections ----------------
    with tc.tile_pool(name="proj_in", bufs=1) as pin, \
         tc.tile_pool(name="ps_qk", bufs=2, space="PSUM") as ps_qk_pool, \
         tc.tile_pool(name="ps_v", bufs=2, space="PSUM") as ps_v_pool:
        xt_sb = pin.tile([128, 8, T], MMDT, tag="xt")
        wqk_sb = pin.tile([128, 8, 512], MMDT, tag="wqk")
        wv_sb = pin.tile([128, 8, 256], MMDT, tag="wv")
        xt_r = xt.rearrange("(c p) t -> p c t", p=128)
        wqk_r = wqk.rearrange("(c p) n -> p c n", p=128)
        wv_r = wv.rearrange("(c p) n -> p c n", p=128)
        for ck in range(8):
            nc.sync.dma_start(out=wqk_sb[:, ck, :], in_=wqk_r[:, ck, :])
            nc.sync.dma_start(out=wv_sb[:, ck, :], in_=wv_r[:, ck, :])
            nc.sync.dma_start(out=xt_sb[:, ck, :], in_=xt_r[:, ck, :])

        # qkT = Wqk^T @ x^T : psum[j, t] per (column slice s, t chunk)
        SLICE_MAP = {0: (0, 0), 1: (1, 0), 2: (0, 1), 3: (1, 1)}  # s -> (hp, qk)
        for s in range(4):
            hp, qk = SLICE_MAP[s]
            for tch in range(NQC):
                ps_qk = ps_qk_pool.tile([128, QCH], F32, tag="psqk")
                for ck in range(8):
                    nc.tensor.matmul(
                        ps_qk,
                        lhsT=wqk_sb[:, ck, s * 128:(s + 1) * 128],
                        rhs=xt_sb[:, ck, tch * QCH:(tch + 1) * QCH],
                        start=(ck == 0), stop=(ck == 7),
                    )
                nc.vector.tensor_scalar_add(
                    out=qk_sb[:, hp, qk, tch * QCH:(tch + 1) * QCH],
                    in0=ps_qk, scalar1=bqk_sb[:, s:s + 1],
                )

        # v = x @ Wv : natural layout, scattered into per-parity column slots
        for kt in range(NKT):
            ps_v = ps_v_pool.tile([128, 256], F32, tag="psv")
            for ck in range(8):
                nc.tensor.matmul(
                    ps_v,
                    lhsT=xt_sb[:, ck, kt * 128:(kt + 1) * 128],
                    rhs=wv_sb[:, ck, :],
                    start=(ck == 0), stop=(ck == 7),
                )
            nc.vector.tensor_copy(
                out=v_sb[:, kt, :, 64:128],
                in_=ps_v.rearrange("p (h d) -> p h d", h=HL))

    # ---------------- phase D: attention per head ----------------
    with tc.tile_pool(name="ps_s", bufs=3, space="PSUM") as pss_pool, \
         tc.tile_pool(name="ps_pv", bufs=1, space="PSUM") as pv_pool, \
         tc.tile_pool(name="st", bufs=4) as st_pool, \
         tc.tile_pool(name="rc", bufs=2) as rc_pool, \
         tc.tile_pool(name="atmp", bufs=2) as atmp_pool:
        for h in range(HL):
            hp, off = h // 2, 64 * (h % 2)
            pv = [pv_pool.tile([128, QCH], F32, tag=f"pv{j}", name=f"pv{j}")
                  for j in range(NQC)]
            for i in range(NKT):
                j0 = i // 4
                for j in range(j0, NQC):
                    lo = i * 128 - j * QCH if j == j0 else 0
                    # f32r matmuls drop to 4 cyc/row below 256 moving cols;
                    # keep the sliced width >= 256
                    lom = min(lo, 256)
                    ps_s = pss_pool.tile([128, QCH], F32, tag="pss")
                    nc.tensor.matmul(
                        ps_s[:, lom:],
                        lhsT=qk_sb[off:off + 64, hp, 1, i * 128:(i + 1) * 128],
                        rhs=qk_sb[off:off + 64, hp, 0,
                                      j * QCH + lom:(j + 1) * QCH],
                        start=True, stop=True,
                    )
                    st = st_pool.tile([128, QCH], MMDT, tag="st")
                    nc.scalar.activation(
                        out=st[:, lom:], in_=ps_s[:, lom:],
                        func=ACT.Exp, scale=0.125,
                    )
                    if j == j0:
                        # causal mask: zero cols [lom:lo) + triangular diag
                        # block [lo:lo+128) in one multiply
                        mask = trix_sb if lo > lom else tri_sb
                        nc.vector.tensor_mul(
                            out=st[:, lom:lo + 128],
                            in0=st[:, lom:lo + 128], in1=mask,
                        )
                    nc.tensor.matmul(
                        pv[j][:, lom:],
                        lhsT=v_sb[:, i, h, :],
                        rhs=st[:, lom:],
                        start=(i == 0), stop=(i == 4 * j + 3),
                    )
            for j in range(NQC):
                rc = rc_pool.tile([128, QCH], F32, tag="rc", name="rc")
                nc.vector.reciprocal_approx_fast(
                    out=rc[0:64, :], in_=pv[j][0:64, :])
                # mirror the reciprocal rows into the attn partition half
                nc.sync.dma_start(out=rc[64:128, :], in_=rc[0:64, :])
                js = slice(j * QCH, (j + 1) * QCH)
                if off == 64:
                    nc.vector.tensor_mul(
                        out=at_sb[64:128, hp, js],
                        in0=pv[j][64:128, :], in1=rc[64:128, :],
                    )
                else:
                    atmp = atmp_pool.tile([128, QCH], MMDT, tag="atmp",
                                          name="atmp")
                    nc.vector.tensor_mul(
                        out=atmp[64:128, :],
                        in0=pv[j][64:128, :], in1=rc[64:128, :],
                    )
                    nc.sync.dma_start(
                        out=at_sb[0:64, hp, js], in_=atmp[64:128, :])

    # ---------------- phase E: output projection ----------------
    with tc.tile_pool(name="ps_y", bufs=2, space="PSUM") as psy_pool, \
         tc.tile_pool(name="yo", bufs=3) as y_pool:
        for tt in range(NKT):
            for n2 in range(2):
                ps_y = psy_pool.tile([128, QCH], F32, tag="psy")
                for ct in range(2):
                    nc.tensor.matmul(
                        ps_y,
                        lhsT=at_sb[:, ct, tt * 128:(tt + 1) * 128],
                        rhs=wp_sb[:, ct, n2 * QCH:(n2 + 1) * QCH],
                        start=(ct == 0), stop=(ct == 1),
                    )
                yt = y_pool.tile([128, QCH], F32, tag="yt")
                nc.scalar.activation(out=yt, in_=ps_y, func=ACT.Copy)
                nc.sync.dma_start(
                    out=y[tt * 128:(tt + 1) * 128, n2 * QCH:(n2 + 1) * QCH],
                    in_=yt,
                )

    if debug_dumps:
        qk_d = nc.dram_tensor("qk_d", [128, 2, 2, T], MMDT, kind="ExternalOutput").ap()
        v_d = nc.dram_tensor("v_d", [128, NKT, HL, 128], MMDT,
                             kind="ExternalOutput").ap()
        at_d = nc.dram_tensor("at_d", [128, 2, T], MMDT, kind="ExternalOutput").ap()
        nc.sync.dma_start(out=qk_d, in_=qk_sb)
        nc.sync.dma_start(out=v_d, in_=v_sb)
        nc.sync.dma_start(out=at_d, in_=at_sb)

    persist.release()


_PROGRAM = None


def build_program(debug_dumps=False):
    global _PROGRAM
    if _PROGRAM is None or debug_dumps:
        nc = bacc.Bacc("TRN2", debug=False, num_devices=N_CORES)
        with tile.TileContext(nc) as tc:
            _body(tc, debug_dumps=debug_dumps)
        nc.compile()
        if debug_dumps:
            return nc
        _PROGRAM = nc
    return _PROGRAM


def make_in_maps(x, W_attn, b_attn, W_proj):
    """Host-side shard: per-core input dict."""
    x = np.asarray(x, np.float32)
    W_attn = np.asarray(W_attn, np.float32)
    b_attn = np.asarray(b_attn, np.float32)
    W_proj = np.asarray(W_proj, np.float32)
    tri = np.triu(np.ones((128, 128), np.float32))  # tri[k, q] = k <= q
    trix = np.concatenate(
        [np.zeros((128, 128), np.float32), tri], axis=1)  # [0 | tri]
    in_maps = []
    for c in range(N_CORES):
        b, g = divmod(c, 4)
        xt = np.ascontiguousarray(x[b].T)  # [C, T]
        q0 = 256 * g
        cols = np.r_[q0:q0 + 256, C + q0:C + q0 + 256]  # q then k, heads 4g..4g+3
        wqk = np.ascontiguousarray(W_attn[:, cols])  # [C, 512] = [q01|q23|k01|k23]
        wv = np.ascontiguousarray(W_attn[:, 2 * C + q0:2 * C + q0 + 256])
        wp_l = np.ascontiguousarray(W_proj[q0:q0 + 256, :])
        bqk = np.ascontiguousarray(
            b_attn[cols].reshape(4, 128).T)  # [128, 4], col s = slice s bias
        in_maps.append({
            "xt": round_f32r(xt), "wqk": round_f32r(wqk),
            "wv": round_f32r(wv), "wp": round_f32r(wp_l),
            "bqk": bqk, "tri": tri, "trix": trix,
        })
    return in_maps


def kernel(x, W_attn, b_attn, W_proj, b_proj):
    global LAST_RESULT
    W_attn = np.asarray(W_attn, np.float32)
    W_proj = np.asarray(W_proj, np.float32)
    b_attn = np.asarray(b_attn, np.float32)
    b_proj = np.asarray(b_proj, np.float32)

    nc = build_program()
    in_maps = make_in_maps(x, W_attn, b_attn, W_proj)
    res = run_bass_kernel_spmd(nc, in_maps, core_ids=list(range(N_CORES)))
    LAST_RESULT = res
    parts = [r["y"] for r in res.results]
    yb = [parts[0] + parts[1] + parts[2] + parts[3],
          parts[4] + parts[5] + parts[6] + parts[7]]
    out = np.stack(yb, axis=0)  # [B, T, C]
    # host-folded biases: b_proj, and the v-part of b_attn (softmax rows sum to 1)
    out += (b_proj + b_attn[2 * C:] @ W_proj)[None, None, :]
    return out.astype(np.float32)


# revision 31
# speedup vs baseline: 1.0953x; 1.0953x over previous
"""Causal self-attention (B=2, T=2048, C=1024, H=16, D=64) on 8 trn2 NeuronCores.

Sharding: core c -> batch b = c // 4, head group g = c % 4 (heads 4g..4g+3).
Each core computes, for its batch and its 4 heads:
    qkT   = Wqk_local^T @ x_b^T          [512, 2048]   (q/k transposed layout)
    v     = x_b @ Wv_local               [2048, 256]   (natural layout)
    sT    = k q^T (per head)             [k, q] blocks; exp(s/8), causal mask
    pv    = (v|ones)^T @ exp(sT)         [128, q]: 64 attn rows + 64 denom rows
    y_par = attnT-contraction @ Wp_local [2048, 1024]
Host: y[b] = sum of the 4 partials + b_proj + (b_attn_v @ W_proj).

The host pre-transposes x (layout choice only - all FLOPs stay on device)
and column/row-shards the weights. b_attn(q,k) folded in via per-partition
activation bias; b_attn(v) and b_proj folded in on the host (exact since
softmax rows sum to 1).

Engine-lane constraint: DVE/ACT operands must share the partition window, so
attention rows live at partitions 0:64 for even heads and 64:128 for odd
heads (the v|ones weight column order flips per parity), and the reciprocal
row block is mirrored across the partition halves with a tiny SBUF->SBUF DMA.
"""

import os
import sys

import numpy as np

for _p in ("/opt/trn_rl_repo",):
    if _p not in sys.path:
        sys.path.insert(0, _p)

import concourse.bass as bass  # noqa: E402,F401
import concourse.mybir as mybir  # noqa: E402
import concourse.tile as tile  # noqa: E402
from concourse import bacc  # noqa: E402
from concourse.bass_utils import run_bass_kernel_spmd  # noqa: E402

B, T, C, H, D = 2, 2048, 1024, 16, 64
HL = 4          # heads per core
N_CORES = 8
QCH = 512       # q-chunk width (one PSUM bank of fp32)
NKT = T // 128  # 16 k-tiles per head
NQC = T // QCH  # 4 q-chunks

F32 = mybir.dt.float32

# matmul compute dtype: "f32" (4 cyc/row) or "f32r" (1 cyc/row for moving
# dim >= 256; fp32 stored with mantissa rounded to 11 bits, ~1.2e-4 rel)
MM_DT = os.environ.get("KMM_DT", "f32r")
KVAR = os.environ.get("KVAR", "C")
MMDT = {"f32": F32, "f32r": mybir.dt.float32r}[MM_DT]

LAST_RESULT = None  # BassKernelResults of the most recent kernel() call


def round_f32r(a):
    """Round-to-nearest-even fp32 -> fp32r (11-bit mantissa, low 12 bits 0)."""
    if MM_DT != "f32r":
        return a
    u = np.ascontiguousarray(a, np.float32).view(np.uint32)
    u = (u + 0x7FF + ((u >> 12) & 1)) & np.uint32(0xFFFFF000)
    return u.view(np.float32)


def _body(tc, debug_dumps=False):
    nc = tc.nc
    ACT = mybir.ActivationFunctionType

    xt = nc.dram_tensor("xt", [C, T], MMDT, kind="ExternalInput").ap()
    wqk = nc.dram_tensor("wqk", [C, 512], MMDT, kind="ExternalInput").ap()
    wv = nc.dram_tensor("wv", [C, 256], MMDT, kind="ExternalInput").ap()
    wp = nc.dram_tensor("wp", [256, C], MMDT, kind="ExternalInput").ap()
    bqk = nc.dram_tensor("bqk", [128, 4], F32, kind="ExternalInput").ap()
    tri = nc.dram_tensor("tri", [128, 128], MMDT, kind="ExternalInput").ap()
    trix = nc.dram_tensor("trix", [128, 256], MMDT, kind="ExternalInput").ap()
    y = nc.dram_tensor("y", [T, C], F32, kind="ExternalOutput").ap()

    # ---------------- persistent SBUF ----------------
    persist = tc.alloc_tile_pool(name="persist", bufs=1)
    qk_sb = persist.tile([128, 2, 2, T], MMDT, tag="qk")    # [p, hpair, q/k, t]
    v_sb = persist.tile([128, NKT, HL, 128], MMDT, tag="v")  # [p, ktile, h, 1|d]
    at_sb = persist.tile([128, 2, T], MMDT, tag="at")       # attnT [p, ctile, t]
    wp_sb = persist.tile([128, 2, C], MMDT, tag="wp")
    bqk_sb = persist.tile([128, 4], F32, tag="bqk")
    tri_sb = persist.tile([128, 128], MMDT, tag="tri")
    trix_sb = persist.tile([128, 256], MMDT, tag="trix")

    nc.sync.dma_start(out=bqk_sb, in_=bqk)
    nc.sync.dma_start(out=tri_sb, in_=tri)
    nc.sync.dma_start(out=trix_sb, in_=trix)
    # ones|v weight layout (all heads): ones cols 0:64 -> denominator rows 0:64
    # of the PV psum; v cols 64:128 -> attn rows 64:128.  (reciprocal_approx
    # is a custom DVE op that only works at partition base 0, so the denom
    # must always land in the low half.)  memset can't write f32r, so the
    # ones come from tri's all-ones last column, free-broadcast by the DVE.
    nc.vector.tensor_copy(
        out=v_sb[:, :, :, 0:64],
        in_=tri_sb[:, 127:128].broadcast_to([128, NKT, HL, 64]))

    # ---------------- load + qkT projection ----------------
    pin = tc.alloc_tile_pool(name="pin", bufs=1)       # xt + wv (live into h0)
    pinw = tc.alloc_tile_pool(name="pinw", bufs=1)     # wqk (load phase only)
    xt_sb = pin.tile([128, 8, T], MMDT, tag="xt")
    wv_sb = pin.tile([128, 8, 256], MMDT, tag="wv")
    wqk_sb = pinw.tile([128, 8, 512], MMDT, tag="wqk")
    xt_r = xt.rearrange("(c p) t -> p c t", p=128)
    wqk_r = wqk.rearrange("(c p) n -> p c n", p=128)
    wv_r = wv.rearrange("(c p) n -> p c n", p=128)
    for ck in range(8):
        nc.sync.dma_start(out=wqk_sb[:, ck, :], in_=wqk_r[:, ck, :])
        for tq in range(NQC):
            nc.sync.dma_start(out=xt_sb[:, ck, tq * QCH:(tq + 1) * QCH],
                              in_=xt_r[:, ck, tq * QCH:(tq + 1) * QCH])
    for ck in range(8):
        nc.sync.dma_start(out=wv_sb[:, ck, :], in_=wv_r[:, ck, :])
    nc.sync.dma_start(out=wp_sb, in_=wp.rearrange("(c p) n -> p c n", p=128))

    # qkT = Wqk^T @ x^T in two ck-outer waves of 8 psum banks so the first
    # wave's matmuls start as each xt chunk's DMA lands.  Wave 1 covers the
    # q and k slices of heads 0/1 so their attention could start earliest.
    SLICE_MAP = {0: (0, 0), 1: (1, 0), 2: (0, 1), 3: (1, 1)}  # s -> (hp, qk)
    ps_qk_pool = tc.alloc_tile_pool(name="ps_qk", bufs=1, space="PSUM")
    wave_plan = ((0, 2), (1, 3)) if KVAR != "C" else ((0, 2),)
    for wave, svals in enumerate(wave_plan):
        ps = {}
        for s in svals:
            for tch in range(NQC):
                ps[s, tch] = ps_qk_pool.tile(
                    [128, QCH], F32, tag=f"psqk{s % 2}_{tch}",
                    name=f"psqk{s}_{tch}")
        for ck in range(8):
            for tch in range(NQC):
                for s in svals:
                    nc.tensor.matmul(
                        ps[s, tch],
                        lhsT=wqk_sb[:, ck, s * 128:(s + 1) * 128],
                        rhs=xt_sb[:, ck, tch * QCH:(tch + 1) * QCH],
                        start=(ck == 0), stop=(ck == 7),
                    )
        for (s, tch), p in ps.items():
            hp, qk = SLICE_MAP[s]
            nc.vector.tensor_scalar_add(
                out=qk_sb[:, hp, qk, tch * QCH:(tch + 1) * QCH],
                in0=p, scalar1=bqk_sb[:, s:s + 1],
            )
    ps_qk_pool.release()
    if KVAR != "C":
        pinw.release()

    # ---------------- attention (v projection runs JIT inside head 0) ------
    # Each head runs in two chunk passes (j in {0,1} then {2,3}): only 2 PV
    # accumulator banks are held at a time, freeing banks for the v matmuls.
    def emit_v(kt, pool, ntag):
        p = pool.tile([128, 256], F32, tag=f"psv{kt % ntag}", name=f"psv{kt}")
        for ck in range(8):
            nc.tensor.matmul(
                p,
                lhsT=xt_sb[:, ck, kt * 128:(kt + 1) * 128],
                rhs=wv_sb[:, ck, :],
                start=(ck == 0), stop=(ck == 7),
            )
        nc.vector.tensor_copy(
            out=v_sb[:, kt, :, 64:128],
            in_=p.rearrange("p (h d) -> p h d", h=HL))

    if KVAR in ("B", "C"):
        # serial v projection, 8 rotating banks
        psv_pool = tc.alloc_tile_pool(name="ps_v", bufs=1, space="PSUM")
        for kt in range(NKT):
            emit_v(kt, psv_pool, 8)
        psv_pool.release()

    pss_pool = tc.alloc_tile_pool(name="ps_s", bufs=2, space="PSUM")
    pv_pool = tc.alloc_tile_pool(name="ps_pv", bufs=1, space="PSUM")
    if KVAR == "A":
        psv_pool = tc.alloc_tile_pool(name="ps_v", bufs=1, space="PSUM")
    st_pool = tc.alloc_tile_pool(name="st", bufs=3)
    rc_pool = tc.alloc_tile_pool(name="rc", bufs=3)
    atmp_pool = tc.alloc_tile_pool(name="atmp", bufs=3)

    def norm(h, pv, j):
        """Normalize chunk j of head h from its finished PV accumulator."""
        hp, off = h // 2, 64 * (h % 2)
        rc = rc_pool.tile([128, QCH], F32, tag="rc", name="rc")
        nc.vector.reciprocal_approx_fast(out=rc[0:64, :], in_=pv[j][0:64, :])
        # mirror the reciprocal rows into the attn partition half
        nc.sync.dma_start(out=rc[64:128, :], in_=rc[0:64, :])
        js = slice(j * QCH, (j + 1) * QCH)
        if off == 64:
            nc.vector.tensor_mul(
                out=at_sb[64:128, hp, js],
                in0=pv[j][64:128, :], in1=rc[64:128, :],
            )
        else:
            atmp = atmp_pool.tile([128, QCH], MMDT, tag="atmp", name="atmp")
            nc.vector.tensor_mul(
                out=atmp[64:128, :],
                in0=pv[j][64:128, :], in1=rc[64:128, :],
            )
            nc.sync.dma_start(out=at_sb[0:64, hp, js], in_=atmp[64:128, :])

    if KVAR in ("A", "C"):
        PASSES = [(0, 2), (2, 4)]
        if KVAR == "C":
            _pvc = [0]

            def pv_tag(j):
                _pvc[0] += 1
                return f"pv{_pvc[0] % 3}"
        else:
            pv_tag = lambda j: f"pv{j % 2}"   # noqa: E731
    else:
        PASSES = [(0, 4)]
        pv_tag = lambda j: f"pv{j}"       # noqa: E731

    if KVAR == "C":
        ps_w2_pool = tc.alloc_tile_pool(name="ps_w2", bufs=1, space="PSUM")
        w2_jobs = [(s, tch) for s in (1, 3) for tch in range(NQC)]
        w2_state = {"tile": None, "ck": 0}

        def emit_w2(piece=2):
            # emit `piece` ck-accumulation matmuls of the current wave-2
            # group; returns True while work remains
            if w2_state["tile"] is None:
                if not w2_jobs:
                    return False
                s, tch = w2_jobs[0]
                w2_state["tile"] = ps_w2_pool.tile(
                    [128, QCH], F32, tag="w2", name=f"w2_{s}_{tch}")
                w2_state["ck"] = 0
            s, tch = w2_jobs[0]
            p = w2_state["tile"]
            for ck in range(w2_state["ck"],
                            min(8, w2_state["ck"] + piece)):
                nc.tensor.matmul(
                    p,
                    lhsT=wqk_sb[:, ck, s * 128:(s + 1) * 128],
                    rhs=xt_sb[:, ck, tch * QCH:(tch + 1) * QCH],
                    start=(ck == 0), stop=(ck == 7),
                )
            w2_state["ck"] += piece
            if w2_state["ck"] >= 8:
                hp, qk = SLICE_MAP[s]
                nc.vector.tensor_scalar_add(
                    out=qk_sb[:, hp, qk, tch * QCH:(tch + 1) * QCH],
                    in0=p, scalar1=bqk_sb[:, s:s + 1],
                )
                w2_jobs.pop(0)
                w2_state["tile"] = None
            return bool(w2_jobs)

    for h in range(HL):
        hp, off = h // 2, 64 * (h % 2)
        for ja, jb in PASSES:
            pv = {j: pv_pool.tile([128, QCH], F32, tag=pv_tag(j),
                                  name=f"pv{j}")
                  for j in range(ja, jb)}
            for i in range(4 * (jb - 1) + 4):  # i <= 4*(jb-1)+3
                if KVAR == "A" and h == 0 and ja == 0 and i < 8:
                    emit_v(2 * i, psv_pool, 2)
                    emit_v(2 * i + 1, psv_pool, 2)
                if KVAR == "C" and h < 3 and w2_jobs:
                    emit_w2(2)
                j0 = i // 4
                lo = i * 128 - j0 * QCH        # diag offset inside chunk j0
                lom = min(lo, 256)             # f32r needs >=256 moving cols
                jlist = [j for j in range(max(ja, j0), jb)]
                for ga in range(0, len(jlist), 2):
                    grp = jlist[ga:ga + 2]
                    ps_s = pss_pool.tile([128, 2, QCH], F32, tag="pss",
                                         name="pss")
                    st = st_pool.tile([128, 2, QCH], MMDT, tag="st", name="st")
                    for b, j in enumerate(grp):
                        lj = lom if j == j0 else 0
                        nc.tensor.matmul(
                            ps_s[:, b, lj:],
                            lhsT=qk_sb[off:off + 64, hp, 1,
                                       i * 128:(i + 1) * 128],
                            rhs=qk_sb[off:off + 64, hp, 0,
                                      j * QCH + lj:(j + 1) * QCH],
                            start=True, stop=True,
                        )
                    lg = lom if grp[0] == j0 else 0
                    nc.scalar.activation(
                        out=st.rearrange("p a b -> p (a b)")[
                            :, lg:len(grp) * QCH],
                        in_=ps_s.rearrange("p a b -> p (a b)")[
                            :, lg:len(grp) * QCH],
                        func=ACT.Exp, scale=0.125,
                    )
                    if grp[0] == j0:
                        # causal mask: zero [lom:lo) + triangular diag block
                        mask = trix_sb if lo > lom else tri_sb
                        nc.vector.tensor_mul(
                            out=st[:, 0, lom:lo + 128],
                            in0=st[:, 0, lom:lo + 128], in1=mask,
                        )
                    for b, j in enumerate(grp):
                        lj = lom if j == j0 else 0
                        nc.tensor.matmul(
                            pv[j][:, lj:],
                            lhsT=v_sb[:, i, h, :],
                            rhs=st[:, b, lj:],
                            start=(i == 0), stop=(i == 4 * j + 3),
                        )
                for j in pv:
                    # chunk j's accumulation ends at i == 4j+3: normalize
                    # immediately so the bank frees while later chunks run
                    if i == 4 * j + 3:
                        norm(h, pv, j)
    rel = [atmp_pool, rc_pool, st_pool]
    if KVAR == "A":
        rel.append(psv_pool)
    if KVAR == "C":
        rel.append(ps_w2_pool)
    rel += [pv_pool, pss_pool]
    for p in rel:
        p.release()
    if KVAR == "C":
        pinw.release()
    pin.release()

    # ---------------- output projection ----------------
    with tc.tile_pool(name="ps_y", bufs=4, space="PSUM") as psy_pool, \
         tc.tile_pool(name="yo", bufs=4) as y_pool:
        for tt in range(NKT):
            for n2 in range(2):
                ps_y = psy_pool.tile([128, QCH], F32, tag="psy")
                for ct in range(2):
                    nc.tensor.matmul(
                        ps_y,
                        lhsT=at_sb[:, ct, tt * 128:(tt + 1) * 128],
                        rhs=wp_sb[:, ct, n2 * QCH:(n2 + 1) * QCH],
                        start=(ct == 0), stop=(ct == 1),
                    )
                yt = y_pool.tile([128, QCH], F32, tag="yt")
                nc.vector.tensor_copy(out=yt, in_=ps_y)
                nc.sync.dma_start(
                    out=y[tt * 128:(tt + 1) * 128, n2 * QCH:(n2 + 1) * QCH],
                    in_=yt,
                )

    if debug_dumps:
        qk_d = nc.dram_tensor("qk_d", [128, 2, 2, T], MMDT, kind="ExternalOutput").ap()
        v_d = nc.dram_tensor("v_d", [128, NKT, HL, 128], MMDT,
                             kind="ExternalOutput").ap()
        at_d = nc.dram_tensor("at_d", [128, 2, T], MMDT, kind="ExternalOutput").ap()
        nc.sync.dma_start(out=qk_d, in_=qk_sb)
        nc.sync.dma_start(out=v_d, in_=v_sb)
        nc.sync.dma_start(out=at_d, in_=at_sb)

    persist.release()


_PROGRAM = None


def build_program(debug_dumps=False):
    global _PROGRAM
    if _PROGRAM is None or debug_dumps:
        nc = bacc.Bacc("TRN2", debug=False, num_devices=N_CORES)
        with tile.TileContext(nc) as tc:
            _body(tc, debug_dumps=debug_dumps)
        nc.compile()
        if debug_dumps:
            return nc
        _PROGRAM = nc
    return _PROGRAM


def make_in_maps(x, W_attn, b_attn, W_proj):
    """Host-side shard: per-core input dict."""
    x = np.asarray(x, np.float32)
    W_attn = np.asarray(W_attn, np.float32)
    b_attn = np.asarray(b_attn, np.float32)
    W_proj = np.asarray(W_proj, np.float32)
    tri = np.triu(np.ones((128, 128), np.float32))  # tri[k, q] = k <= q
    trix = np.concatenate(
        [np.zeros((128, 128), np.float32), tri], axis=1)  # [0 | tri]
    in_maps = []
    for c in range(N_CORES):
        b, g = divmod(c, 4)
        xt = np.ascontiguousarray(x[b].T)  # [C, T]
        q0 = 256 * g
        cols = np.r_[q0:q0 + 256, C + q0:C + q0 + 256]  # q then k, heads 4g..4g+3
        wqk = np.ascontiguousarray(W_attn[:, cols])  # [C, 512] = [q01|q23|k01|k23]
        wv = np.ascontiguousarray(W_attn[:, 2 * C + q0:2 * C + q0 + 256])
        wp_l = np.ascontiguousarray(W_proj[q0:q0 + 256, :])
        bqk = np.ascontiguousarray(
            b_attn[cols].reshape(4, 128).T)  # [128, 4], col s = slice s bias
        in_maps.append({
            "xt": round_f32r(xt), "wqk": round_f32r(wqk),
            "wv": round_f32r(wv), "wp": round_f32r(wp_l),
            "bqk": bqk, "tri": tri, "trix": trix,
        })
    return in_maps


def kernel(x, W_attn, b_attn, W_proj, b_proj):
    global LAST_RESULT
    W_attn = np.asarray(W_attn, np.float32)
    W_proj = np.asarray(W_proj, np.float32)
    b_attn = np.asarray(b_attn, np.float32)
    b_proj = np.asarray(b_proj, np.float32)

    nc = build_program()
    in_maps = make_in_maps(x, W_attn, b_attn, W_proj)
    res = run_bass_kernel_spmd(nc, in_maps, core_ids=list(range(N_CORES)))
    LAST_RESULT = res
    parts = [r["y"] for r in res.results]
    yb = [parts[0] + parts[1] + parts[2] + parts[3],
          parts[4] + parts[5] + parts[6] + parts[7]]
    out = np.stack(yb, axis=0)  # [B, T, C]
    # host-folded biases: b_proj, and the v-part of b_attn (softmax rows sum to 1)
    out += (b_proj + b_attn[2 * C:] @ W_proj)[None, None, :]
    return out.astype(np.float32)


# revision 32
# speedup vs baseline: 17869.4806x; 16315.2246x over previous
"""Causal self-attention (B=2, T=2048, C=1024, H=16, D=64) on 8 trn2 NeuronCores.

Sharding: core c -> batch b = c // 4, head group g = c % 4 (heads 4g..4g+3).
Each core computes, for its batch and its 4 heads:
    qkT   = Wqk_local^T @ x_b^T          [512, 2048]   (q/k transposed layout)
    v     = x_b @ Wv_local               [2048, 256]   (natural layout)
    sT    = k q^T (per head)             [k, q] blocks; exp(s/8), causal mask
    pv    = (v|ones)^T @ exp(sT)         [128, q]: 64 attn rows + 64 denom rows
    y_par = attnT-contraction @ Wp_local [2048, 1024]
Host: y[b] = sum of the 4 partials + b_proj + (b_attn_v @ W_proj).

The host pre-transposes x (layout choice only - all FLOPs stay on device)
and column/row-shards the weights. b_attn(q,k) folded in via per-partition
activation bias; b_attn(v) and b_proj folded in on the host (exact since
softmax rows sum to 1).

Engine-lane constraint: DVE/ACT operands must share the partition window, so
attention rows live at partitions 0:64 for even heads and 64:128 for odd
heads (the v|ones weight column order flips per parity), and the reciprocal
row block is mirrored across the partition halves with a tiny SBUF->SBUF DMA.
"""

import os
import sys

import numpy as np

try:
    import concourse.bass  # noqa: F401
except ImportError:
    for _p in ("/opt/trn_rl_repo", "/root/.axon_site/_ro/trn_rl_repo"):
        if os.path.isdir(_p) and _p not in sys.path:
            sys.path.insert(0, _p)

import concourse.bass as bass  # noqa: E402,F401
import concourse.mybir as mybir  # noqa: E402
import concourse.tile as tile  # noqa: E402
from concourse import bacc  # noqa: E402
from concourse.bass_utils import run_bass_kernel_spmd  # noqa: E402

B, T, C, H, D = 2, 2048, 1024, 16, 64
HL = 4          # heads per core
N_CORES = 8
QCH = 512       # q-chunk width (one PSUM bank of fp32)
NKT = T // 128  # 16 k-tiles per head
NQC = T // QCH  # 4 q-chunks

F32 = mybir.dt.float32

# matmul compute dtype: "f32" (4 cyc/row) or "f32r" (1 cyc/row for moving
# dim >= 256; fp32 stored with mantissa rounded to 11 bits, ~1.2e-4 rel)
MM_DT = os.environ.get("KMM_DT", "f32r")
KVAR = os.environ.get("KVAR", "C")
MMDT = {"f32": F32, "f32r": mybir.dt.float32r}[MM_DT]

LAST_RESULT = None  # BassKernelResults of the most recent kernel() call


def round_f32r(a):
    """Round-to-nearest-even fp32 -> fp32r (11-bit mantissa, low 12 bits 0)."""
    if MM_DT != "f32r":
        return a
    u = np.ascontiguousarray(a, np.float32).view(np.uint32)
    u = (u + 0x7FF + ((u >> 12) & 1)) & np.uint32(0xFFFFF000)
    return u.view(np.float32)


def _body(tc, debug_dumps=False):
    nc = tc.nc
    ACT = mybir.ActivationFunctionType

    xt = nc.dram_tensor("xt", [C, T], MMDT, kind="ExternalInput").ap()
    wqk = nc.dram_tensor("wqk", [C, 512], MMDT, kind="ExternalInput").ap()
    wv = nc.dram_tensor("wv", [C, 256], MMDT, kind="ExternalInput").ap()
    wp = nc.dram_tensor("wp", [256, C], MMDT, kind="ExternalInput").ap()
    bqk = nc.dram_tensor("bqk", [128, 4], F32, kind="ExternalInput").ap()
    tri = nc.dram_tensor("tri", [128, 128], MMDT, kind="ExternalInput").ap()
    trix = nc.dram_tensor("trix", [128, 256], MMDT, kind="ExternalInput").ap()
    y = nc.dram_tensor("y", [T, C], F32, kind="ExternalOutput").ap()

    # ---------------- persistent SBUF ----------------
    persist = tc.alloc_tile_pool(name="persist", bufs=1)
    qk_sb = persist.tile([128, 2, 2, T], MMDT, tag="qk")    # [p, hpair, q/k, t]
    v_sb = persist.tile([128, NKT, HL, 128], MMDT, tag="v")  # [p, ktile, h, 1|d]
    at_sb = persist.tile([128, 2, T], MMDT, tag="at")       # attnT [p, ctile, t]
    wp_sb = persist.tile([128, 2, C], MMDT, tag="wp")
    bqk_sb = persist.tile([128, 4], F32, tag="bqk")
    tri_sb = persist.tile([128, 128], MMDT, tag="tri")
    trix_sb = persist.tile([128, 256], MMDT, tag="trix")

    nc.sync.dma_start(out=bqk_sb, in_=bqk)
    nc.sync.dma_start(out=tri_sb, in_=tri)
    nc.sync.dma_start(out=trix_sb, in_=trix)
    # ones|v weight layout (all heads): ones cols 0:64 -> denominator rows 0:64
    # of the PV psum; v cols 64:128 -> attn rows 64:128.  (reciprocal_approx
    # is a custom DVE op that only works at partition base 0, so the denom
    # must always land in the low half.)  memset can't write f32r, so the
    # ones come from tri's all-ones last column, free-broadcast by the DVE.
    nc.vector.tensor_copy(
        out=v_sb[:, :, :, 0:64],
        in_=tri_sb[:, 127:128].broadcast_to([128, NKT, HL, 64]))

    # ---------------- load + qkT projection ----------------
    pin = tc.alloc_tile_pool(name="pin", bufs=1)       # xt + wv (live into h0)
    pinw = tc.alloc_tile_pool(name="pinw", bufs=1)     # wqk (load phase only)
    xt_sb = pin.tile([128, 8, T], MMDT, tag="xt")
    wv_sb = pin.tile([128, 8, 256], MMDT, tag="wv")
    wqk_sb = pinw.tile([128, 8, 512], MMDT, tag="wqk")
    xt_r = xt.rearrange("(c p) t -> p c t", p=128)
    wqk_r = wqk.rearrange("(c p) n -> p c n", p=128)
    wv_r = wv.rearrange("(c p) n -> p c n", p=128)
    for ck in range(8):
        nc.sync.dma_start(out=wqk_sb[:, ck, :], in_=wqk_r[:, ck, :])
        for tq in range(NQC):
            nc.sync.dma_start(out=xt_sb[:, ck, tq * QCH:(tq + 1) * QCH],
                              in_=xt_r[:, ck, tq * QCH:(tq + 1) * QCH])
    for ck in range(8):
        nc.sync.dma_start(out=wv_sb[:, ck, :], in_=wv_r[:, ck, :])
    nc.sync.dma_start(out=wp_sb, in_=wp.rearrange("(c p) n -> p c n", p=128))

    # qkT = Wqk^T @ x^T in two ck-outer waves of 8 psum banks so the first
    # wave's matmuls start as each xt chunk's DMA lands.  Wave 1 covers the
    # q and k slices of heads 0/1 so their attention could start earliest.
    SLICE_MAP = {0: (0, 0), 1: (1, 0), 2: (0, 1), 3: (1, 1)}  # s -> (hp, qk)
    ps_qk_pool = tc.alloc_tile_pool(name="ps_qk", bufs=1, space="PSUM")
    wave_plan = ((0, 2), (1, 3)) if KVAR != "C" else ((0, 2),)
    for wave, svals in enumerate(wave_plan):
        ps = {}
        for s in svals:
            for tch in range(NQC):
                ps[s, tch] = ps_qk_pool.tile(
                    [128, QCH], F32, tag=f"psqk{s % 2}_{tch}",
                    name=f"psqk{s}_{tch}")
        for ck in range(8):
            for tch in range(NQC):
                for s in svals:
                    nc.tensor.matmul(
                        ps[s, tch],
                        lhsT=wqk_sb[:, ck, s * 128:(s + 1) * 128],
                        rhs=xt_sb[:, ck, tch * QCH:(tch + 1) * QCH],
                        start=(ck == 0), stop=(ck == 7),
                    )
        for (s, tch), p in ps.items():
            hp, qk = SLICE_MAP[s]
            nc.vector.tensor_scalar_add(
                out=qk_sb[:, hp, qk, tch * QCH:(tch + 1) * QCH],
                in0=p, scalar1=bqk_sb[:, s:s + 1],
            )
    ps_qk_pool.release()
    if KVAR != "C":
        pinw.release()

    # ---------------- attention (v projection runs JIT inside head 0) ------
    # Each head runs in two chunk passes (j in {0,1} then {2,3}): only 2 PV
    # accumulator banks are held at a time, freeing banks for the v matmuls.
    def emit_v(kt, pool, ntag):
        p = pool.tile([128, 256], F32, tag=f"psv{kt % ntag}", name=f"psv{kt}")
        for ck in range(8):
            nc.tensor.matmul(
                p,
                lhsT=xt_sb[:, ck, kt * 128:(kt + 1) * 128],
                rhs=wv_sb[:, ck, :],
                start=(ck == 0), stop=(ck == 7),
            )
        nc.vector.tensor_copy(
            out=v_sb[:, kt, :, 64:128],
            in_=p.rearrange("p (h d) -> p h d", h=HL))

    if KVAR in ("B", "C"):
        # serial v projection, 8 rotating banks
        psv_pool = tc.alloc_tile_pool(name="ps_v", bufs=1, space="PSUM")
        for kt in range(NKT):
            emit_v(kt, psv_pool, 8)
        psv_pool.release()

    pss_pool = tc.alloc_tile_pool(name="ps_s", bufs=2, space="PSUM")
    pv_pool = tc.alloc_tile_pool(name="ps_pv", bufs=1, space="PSUM")
    if KVAR == "A":
        psv_pool = tc.alloc_tile_pool(name="ps_v", bufs=1, space="PSUM")
    st_pool = tc.alloc_tile_pool(name="st", bufs=3)
    rc_pool = tc.alloc_tile_pool(name="rc", bufs=3)
    atmp_pool = tc.alloc_tile_pool(name="atmp", bufs=3)

    def norm(h, pv, j):
        """Normalize chunk j of head h from its finished PV accumulator."""
        hp, off = h // 2, 64 * (h % 2)
        rc = rc_pool.tile([128, QCH], F32, tag="rc", name="rc")
        nc.vector.reciprocal_approx_fast(out=rc[0:64, :], in_=pv[j][0:64, :])
        # mirror the reciprocal rows into the attn partition half
        nc.sync.dma_start(out=rc[64:128, :], in_=rc[0:64, :])
        js = slice(j * QCH, (j + 1) * QCH)
        if off == 64:
            nc.vector.tensor_mul(
                out=at_sb[64:128, hp, js],
                in0=pv[j][64:128, :], in1=rc[64:128, :],
            )
        else:
            atmp = atmp_pool.tile([128, QCH], MMDT, tag="atmp", name="atmp")
            nc.vector.tensor_mul(
                out=atmp[64:128, :],
                in0=pv[j][64:128, :], in1=rc[64:128, :],
            )
            nc.sync.dma_start(out=at_sb[0:64, hp, js], in_=atmp[64:128, :])

    if KVAR in ("A", "C"):
        PASSES = [(0, 2), (2, 4)]
        if KVAR == "C":
            _pvc = [0]

            def pv_tag(j):
                _pvc[0] += 1
                return f"pv{_pvc[0] % 3}"
        else:
            pv_tag = lambda j: f"pv{j % 2}"   # noqa: E731
    else:
        PASSES = [(0, 4)]
        pv_tag = lambda j: f"pv{j}"       # noqa: E731

    if KVAR == "C":
        ps_w2_pool = tc.alloc_tile_pool(name="ps_w2", bufs=1, space="PSUM")
        w2_jobs = [(s, tch) for s in (1, 3) for tch in range(NQC)]
        w2_state = {"tile": None, "ck": 0}

        def emit_w2(piece=2):
            # emit `piece` ck-accumulation matmuls of the current wave-2
            # group; returns True while work remains
            if w2_state["tile"] is None:
                if not w2_jobs:
                    return False
                s, tch = w2_jobs[0]
                w2_state["tile"] = ps_w2_pool.tile(
                    [128, QCH], F32, tag="w2", name=f"w2_{s}_{tch}")
                w2_state["ck"] = 0
            s, tch = w2_jobs[0]
            p = w2_state["tile"]
            for ck in range(w2_state["ck"],
                            min(8, w2_state["ck"] + piece)):
                nc.tensor.matmul(
                    p,
                    lhsT=wqk_sb[:, ck, s * 128:(s + 1) * 128],
                    rhs=xt_sb[:, ck, tch * QCH:(tch + 1) * QCH],
                    start=(ck == 0), stop=(ck == 7),
                )
            w2_state["ck"] += piece
            if w2_state["ck"] >= 8:
                hp, qk = SLICE_MAP[s]
                nc.vector.tensor_scalar_add(
                    out=qk_sb[:, hp, qk, tch * QCH:(tch + 1) * QCH],
                    in0=p, scalar1=bqk_sb[:, s:s + 1],
                )
                w2_jobs.pop(0)
                w2_state["tile"] = None
            return bool(w2_jobs)

    for h in range(HL):
        hp, off = h // 2, 64 * (h % 2)
        for ja, jb in PASSES:
            pv = {j: pv_pool.tile([128, QCH], F32, tag=pv_tag(j),
                                  name=f"pv{j}")
                  for j in range(ja, jb)}
            for i in range(4 * (jb - 1) + 4):  # i <= 4*(jb-1)+3
                if KVAR == "A" and h == 0 and ja == 0 and i < 8:
                    emit_v(2 * i, psv_pool, 2)
                    emit_v(2 * i + 1, psv_pool, 2)
                if KVAR == "C" and h < 3 and w2_jobs:
                    emit_w2(2)
                j0 = i // 4
                lo = i * 128 - j0 * QCH        # diag offset inside chunk j0
                lom = min(lo, 256)             # f32r needs >=256 moving cols
                jlist = [j for j in range(max(ja, j0), jb)]
                for ga in range(0, len(jlist), 2):
                    grp = jlist[ga:ga + 2]
                    ps_s = pss_pool.tile([128, 2, QCH], F32, tag="pss",
                                         name="pss")
                    st = st_pool.tile([128, 2, QCH], MMDT, tag="st", name="st")
                    for b, j in enumerate(grp):
                        lj = lom if j == j0 else 0
                        nc.tensor.matmul(
                            ps_s[:, b, lj:],
                            lhsT=qk_sb[off:off + 64, hp, 1,
                                       i * 128:(i + 1) * 128],
                            rhs=qk_sb[off:off + 64, hp, 0,
                                      j * QCH + lj:(j + 1) * QCH],
                            start=True, stop=True,
                        )
                    lg = lom if grp[0] == j0 else 0
                    nc.scalar.activation(
                        out=st.rearrange("p a b -> p (a b)")[
                            :, lg:len(grp) * QCH],
                        in_=ps_s.rearrange("p a b -> p (a b)")[
                            :, lg:len(grp) * QCH],
                        func=ACT.Exp, scale=0.125,
                    )
                    if grp[0] == j0:
                        # causal mask: zero [lom:lo) + triangular diag block
                        mask = trix_sb if lo > lom else tri_sb
                        nc.vector.tensor_mul(
                            out=st[:, 0, lom:lo + 128],
                            in0=st[:, 0, lom:lo + 128], in1=mask,
                        )
                    for b, j in enumerate(grp):
                        lj = lom if j == j0 else 0
                        nc.tensor.matmul(
                            pv[j][:, lj:],
                            lhsT=v_sb[:, i, h, :],
                            rhs=st[:, b, lj:],
                            start=(i == 0), stop=(i == 4 * j + 3),
                        )
                for j in pv:
                    # chunk j's accumulation ends at i == 4j+3: normalize
                    # immediately so the bank frees while later chunks run
                    if i == 4 * j + 3:
                        norm(h, pv, j)
    rel = [atmp_pool, rc_pool, st_pool]
    if KVAR == "A":
        rel.append(psv_pool)
    if KVAR == "C":
        rel.append(ps_w2_pool)
    rel += [pv_pool, pss_pool]
    for p in rel:
        p.release()
    if KVAR == "C":
        pinw.release()
    pin.release()

    # ---------------- output projection ----------------
    with tc.tile_pool(name="ps_y", bufs=4, space="PSUM") as psy_pool, \
         tc.tile_pool(name="yo", bufs=4) as y_pool:
        for tt in range(NKT):
            for n2 in range(2):
                ps_y = psy_pool.tile([128, QCH], F32, tag="psy")
                for ct in range(2):
                    nc.tensor.matmul(
                        ps_y,
                        lhsT=at_sb[:, ct, tt * 128:(tt + 1) * 128],
                        rhs=wp_sb[:, ct, n2 * QCH:(n2 + 1) * QCH],
                        start=(ct == 0), stop=(ct == 1),
                    )
                yt = y_pool.tile([128, QCH], F32, tag="yt")
                nc.vector.tensor_copy(out=yt, in_=ps_y)
                nc.sync.dma_start(
                    out=y[tt * 128:(tt + 1) * 128, n2 * QCH:(n2 + 1) * QCH],
                    in_=yt,
                )

    if debug_dumps:
        qk_d = nc.dram_tensor("qk_d", [128, 2, 2, T], MMDT, kind="ExternalOutput").ap()
        v_d = nc.dram_tensor("v_d", [128, NKT, HL, 128], MMDT,
                             kind="ExternalOutput").ap()
        at_d = nc.dram_tensor("at_d", [128, 2, T], MMDT, kind="ExternalOutput").ap()
        nc.sync.dma_start(out=qk_d, in_=qk_sb)
        nc.sync.dma_start(out=v_d, in_=v_sb)
        nc.sync.dma_start(out=at_d, in_=at_sb)

    persist.release()


_PROGRAM = None


def build_program(debug_dumps=False):
    global _PROGRAM
    if _PROGRAM is None or debug_dumps:
        nc = bacc.Bacc("TRN2", debug=False, num_devices=N_CORES)
        with tile.TileContext(nc) as tc:
            _body(tc, debug_dumps=debug_dumps)
        nc.compile()
        if debug_dumps:
            return nc
        _PROGRAM = nc
    return _PROGRAM


def make_in_maps(x, W_attn, b_attn, W_proj):
    """Host-side shard: per-core input dict."""
    x = np.asarray(x, np.float32)
    W_attn = np.asarray(W_attn, np.float32)
    b_attn = np.asarray(b_attn, np.float32)
    W_proj = np.asarray(W_proj, np.float32)
    tri = np.triu(np.ones((128, 128), np.float32))  # tri[k, q] = k <= q
    trix = np.concatenate(
        [np.zeros((128, 128), np.float32), tri], axis=1)  # [0 | tri]
    in_maps = []
    for c in range(N_CORES):
        b, g = divmod(c, 4)
        xt = np.ascontiguousarray(x[b].T)  # [C, T]
        q0 = 256 * g
        cols = np.r_[q0:q0 + 256, C + q0:C + q0 + 256]  # q then k, heads 4g..4g+3
        wqk = np.ascontiguousarray(W_attn[:, cols])  # [C, 512] = [q01|q23|k01|k23]
        wv = np.ascontiguousarray(W_attn[:, 2 * C + q0:2 * C + q0 + 256])
        wp_l = np.ascontiguousarray(W_proj[q0:q0 + 256, :])
        bqk = np.ascontiguousarray(
            b_attn[cols].reshape(4, 128).T)  # [128, 4], col s = slice s bias
        in_maps.append({
            "xt": round_f32r(xt), "wqk": round_f32r(wqk),
            "wv": round_f32r(wv), "wp": round_f32r(wp_l),
            "bqk": bqk, "tri": tri, "trix": trix,
        })
    return in_maps


def kernel(x, W_attn, b_attn, W_proj, b_proj):
    global LAST_RESULT
    W_attn = np.asarray(W_attn, np.float32)
    W_proj = np.asarray(W_proj, np.float32)
    b_attn = np.asarray(b_attn, np.float32)
    b_proj = np.asarray(b_proj, np.float32)

    nc = build_program()
    in_maps = make_in_maps(x, W_attn, b_attn, W_proj)
    res = run_bass_kernel_spmd(nc, in_maps, core_ids=list(range(N_CORES)))
    LAST_RESULT = res
    parts = [r["y"] for r in res.results]
    yb = [parts[0] + parts[1] + parts[2] + parts[3],
          parts[4] + parts[5] + parts[6] + parts[7]]
    out = np.stack(yb, axis=0)  # [B, T, C]
    # host-folded biases: b_proj, and the v-part of b_attn (softmax rows sum to 1)
    out += (b_proj + b_attn[2 * C:] @ W_proj)[None, None, :]
    return out.astype(np.float32)


# revision 35
# speedup vs baseline: 18125.7090x; 1.0143x over previous
"""Causal self-attention (B=2, T=2048, C=1024, H=16, D=64) on 8 trn2 NeuronCores.

Sharding: core c -> batch b = c // 4, head group g = c % 4 (heads 4g..4g+3).
Each core computes, for its batch and its 4 heads:
    qkT   = Wqk_local^T @ x_b^T          [512, 2048]   (q/k transposed layout)
    v     = x_b @ Wv_local               [2048, 256]   (natural layout)
    sT    = k q^T (per head)             [k, q] blocks; exp(s/8), causal mask
    pv    = (v|ones)^T @ exp(sT)         [128, q]: 64 attn rows + 64 denom rows
    y_par = attnT-contraction @ Wp_local [2048, 1024]
Host: y[b] = sum of the 4 partials + b_proj + (b_attn_v @ W_proj).

The host pre-transposes x (layout choice only - all FLOPs stay on device)
and column/row-shards the weights. b_attn(q,k) folded in via per-partition
activation bias; b_attn(v) and b_proj folded in on the host (exact since
softmax rows sum to 1).

Engine-lane constraint: DVE/ACT operands must share the partition window, so
attention rows live at partitions 0:64 for even heads and 64:128 for odd
heads (the v|ones weight column order flips per parity), and the reciprocal
row block is mirrored across the partition halves with a tiny SBUF->SBUF DMA.
"""

import os
import sys

import numpy as np

try:
    import concourse.bass  # noqa: F401
except ImportError:
    for _p in ("/opt/trn_rl_repo", "/root/.axon_site/_ro/trn_rl_repo"):
        if os.path.isdir(_p) and _p not in sys.path:
            sys.path.insert(0, _p)

import concourse.bass as bass  # noqa: E402,F401
import concourse.mybir as mybir  # noqa: E402
import concourse.tile as tile  # noqa: E402
from concourse import bacc  # noqa: E402
from concourse.bass_utils import run_bass_kernel_spmd  # noqa: E402

B, T, C, H, D = 2, 2048, 1024, 16, 64
HL = 4          # heads per core
N_CORES = 8
QCH = 512       # q-chunk width (one PSUM bank of fp32)
NKT = T // 128  # 16 k-tiles per head
NQC = T // QCH  # 4 q-chunks

F32 = mybir.dt.float32

# matmul compute dtype: "f32" (4 cyc/row) or "f32r" (1 cyc/row for moving
# dim >= 256; fp32 stored with mantissa rounded to 11 bits, ~1.2e-4 rel)
MM_DT = os.environ.get("KMM_DT", "f32r")
KVAR = os.environ.get("KVAR", "D")
MMDT = {"f32": F32, "f32r": mybir.dt.float32r}[MM_DT]

LAST_RESULT = None  # BassKernelResults of the most recent kernel() call


def round_f32r(a):
    """Round-to-nearest-even fp32 -> fp32r (11-bit mantissa, low 12 bits 0)."""
    if MM_DT != "f32r":
        return a
    u = np.ascontiguousarray(a, np.float32).view(np.uint32)
    u = (u + 0x7FF + ((u >> 12) & 1)) & np.uint32(0xFFFFF000)
    return u.view(np.float32)


def _body(tc, debug_dumps=False):
    nc = tc.nc
    ACT = mybir.ActivationFunctionType

    xt = nc.dram_tensor("xt", [C, T], MMDT, kind="ExternalInput").ap()
    wqk = nc.dram_tensor("wqk", [C, 512], MMDT, kind="ExternalInput").ap()
    wv = nc.dram_tensor("wv", [C, 256], MMDT, kind="ExternalInput").ap()
    wp = nc.dram_tensor("wp", [256, C], MMDT, kind="ExternalInput").ap()
    bqk = nc.dram_tensor("bqk", [128, 4], F32, kind="ExternalInput").ap()
    tri = nc.dram_tensor("tri", [128, 128], MMDT, kind="ExternalInput").ap()
    trix = nc.dram_tensor("trix", [128, 256], MMDT, kind="ExternalInput").ap()
    y = nc.dram_tensor("y", [T, C], F32, kind="ExternalOutput").ap()

    # ---------------- persistent SBUF ----------------
    persist = tc.alloc_tile_pool(name="persist", bufs=1)
    qk_sb = persist.tile([128, 2, 2, T], MMDT, tag="qk")    # [p, hpair, q/k, t]
    v_sb = persist.tile([128, NKT, HL, 128], MMDT, tag="v")  # [p, ktile, h, 1|d]
    at_sb = persist.tile([128, 2, T], MMDT, tag="at")       # attnT [p, ctile, t]
    wp_sb = persist.tile([128, 2, C], MMDT, tag="wp")
    bqk_sb = persist.tile([128, 4], F32, tag="bqk")
    tri_sb = persist.tile([128, 128], MMDT, tag="tri")
    trix_sb = persist.tile([128, 256], MMDT, tag="trix")

    nc.sync.dma_start(out=bqk_sb, in_=bqk)
    nc.sync.dma_start(out=tri_sb, in_=tri)
    nc.sync.dma_start(out=trix_sb, in_=trix)
    # ones|v weight layout (all heads): ones cols 0:64 -> denominator rows 0:64
    # of the PV psum; v cols 64:128 -> attn rows 64:128.  (reciprocal_approx
    # is a custom DVE op that only works at partition base 0, so the denom
    # must always land in the low half.)  memset can't write f32r, so the
    # ones come from tri's all-ones last column, free-broadcast by the DVE.
    nc.vector.tensor_copy(
        out=v_sb[:, :, :, 0:64],
        in_=tri_sb[:, 127:128].broadcast_to([128, NKT, HL, 64]))

    # ---------------- load + qkT projection ----------------
    pin = tc.alloc_tile_pool(name="pin", bufs=1)       # xt + wv (live into h0)
    pinw = tc.alloc_tile_pool(name="pinw", bufs=1)     # wqk (load phase only)
    xt_sb = pin.tile([128, 8, T], MMDT, tag="xt")
    wv_sb = pin.tile([128, 8, 256], MMDT, tag="wv")
    wqk_sb = pinw.tile([128, 8, 512], MMDT, tag="wqk")
    xt_r = xt.rearrange("(c p) t -> p c t", p=128)
    wqk_r = wqk.rearrange("(c p) n -> p c n", p=128)
    wv_r = wv.rearrange("(c p) n -> p c n", p=128)
    for ck in range(8):
        nc.sync.dma_start(out=wqk_sb[:, ck, :], in_=wqk_r[:, ck, :])
        for tq in range(NQC):
            nc.sync.dma_start(out=xt_sb[:, ck, tq * QCH:(tq + 1) * QCH],
                              in_=xt_r[:, ck, tq * QCH:(tq + 1) * QCH])
    for ck in range(8):
        nc.sync.dma_start(out=wv_sb[:, ck, :], in_=wv_r[:, ck, :])
    nc.sync.dma_start(out=wp_sb, in_=wp.rearrange("(c p) n -> p c n", p=128))

    # qkT = Wqk^T @ x^T in two ck-outer waves of 8 psum banks so the first
    # wave's matmuls start as each xt chunk's DMA lands.  Wave 1 covers the
    # q and k slices of heads 0/1 so their attention could start earliest.
    SLICE_MAP = {0: (0, 0), 1: (1, 0), 2: (0, 1), 3: (1, 1)}  # s -> (hp, qk)
    ps_qk_pool = tc.alloc_tile_pool(name="ps_qk", bufs=1, space="PSUM")
    wave_plan = ((0, 2),) if KVAR in ("C", "D") else ((0, 2), (1, 3))
    for wave, svals in enumerate(wave_plan):
        ps = {}
        for s in svals:
            for tch in range(NQC):
                ps[s, tch] = ps_qk_pool.tile(
                    [128, QCH], F32, tag=f"psqk{s % 2}_{tch}",
                    name=f"psqk{s}_{tch}")
        for ck in range(8):
            for tch in range(NQC):
                for s in svals:
                    nc.tensor.matmul(
                        ps[s, tch],
                        lhsT=wqk_sb[:, ck, s * 128:(s + 1) * 128],
                        rhs=xt_sb[:, ck, tch * QCH:(tch + 1) * QCH],
                        start=(ck == 0), stop=(ck == 7),
                    )
        for (s, tch), p in ps.items():
            hp, qk = SLICE_MAP[s]
            nc.vector.tensor_scalar_add(
                out=qk_sb[:, hp, qk, tch * QCH:(tch + 1) * QCH],
                in0=p, scalar1=bqk_sb[:, s:s + 1],
            )
    ps_qk_pool.release()
    if KVAR not in ("C", "D"):
        pinw.release()

    # ---------------- attention (v projection runs JIT inside head 0) ------
    # Each head runs in two chunk passes (j in {0,1} then {2,3}): only 2 PV
    # accumulator banks are held at a time, freeing banks for the v matmuls.
    def emit_v(kt, pool, ntag):
        p = pool.tile([128, 256], F32, tag=f"psv{kt % ntag}", name=f"psv{kt}")
        for ck in range(8):
            nc.tensor.matmul(
                p,
                lhsT=xt_sb[:, ck, kt * 128:(kt + 1) * 128],
                rhs=wv_sb[:, ck, :],
                start=(ck == 0), stop=(ck == 7),
            )
        nc.vector.tensor_copy(
            out=v_sb[:, kt, :, 64:128],
            in_=p.rearrange("p (h d) -> p h d", h=HL))

    if KVAR in ("B", "C"):
        # serial v projection, 8 rotating banks
        psv_pool = tc.alloc_tile_pool(name="ps_v", bufs=1, space="PSUM")
        for kt in range(NKT):
            emit_v(kt, psv_pool, 8)
        psv_pool.release()

    pss_pool = tc.alloc_tile_pool(name="ps_s", bufs=2, space="PSUM")
    pv_pool = tc.alloc_tile_pool(name="ps_pv", bufs=1, space="PSUM")
    if KVAR == "A":
        psv_pool = tc.alloc_tile_pool(name="ps_v", bufs=1, space="PSUM")
    st_pool = tc.alloc_tile_pool(name="st", bufs=3)
    rc_pool = tc.alloc_tile_pool(name="rc", bufs=3)
    atmp_pool = tc.alloc_tile_pool(name="atmp", bufs=3)

    def norm(h, pv, j):
        """Normalize chunk j of head h from its finished PV accumulator."""
        hp, off = h // 2, 64 * (h % 2)
        rc = rc_pool.tile([128, QCH], F32, tag="rc", name="rc")
        nc.vector.reciprocal_approx_fast(out=rc[0:64, :], in_=pv[j][0:64, :])
        # mirror the reciprocal rows into the attn partition half
        nc.sync.dma_start(out=rc[64:128, :], in_=rc[0:64, :])
        js = slice(j * QCH, (j + 1) * QCH)
        if off == 64:
            nc.vector.tensor_mul(
                out=at_sb[64:128, hp, js],
                in0=pv[j][64:128, :], in1=rc[64:128, :],
            )
        else:
            atmp = atmp_pool.tile([128, QCH], MMDT, tag="atmp", name="atmp")
            nc.vector.tensor_mul(
                out=atmp[64:128, :],
                in0=pv[j][64:128, :], in1=rc[64:128, :],
            )
            nc.sync.dma_start(out=at_sb[0:64, hp, js], in_=atmp[64:128, :])

    if KVAR in ("A", "C", "D"):
        PASSES = [(0, 2), (2, 4)]
        if KVAR in ("C", "D"):
            _pvc = [0]

            def pv_tag(j):
                _pvc[0] += 1
                return f"pv{_pvc[0] % 3}"
        else:
            pv_tag = lambda j: f"pv{j % 2}"   # noqa: E731
    else:
        PASSES = [(0, 4)]
        pv_tag = lambda j: f"pv{j}"       # noqa: E731

    if KVAR in ("C", "D"):
        ps_w2_pool = tc.alloc_tile_pool(name="ps_w2", bufs=1, space="PSUM")
        w2_jobs = [(s, tch) for s in (1, 3) for tch in range(NQC)]
        w2_state = {"tile": None, "ck": 0}

        def emit_w2(piece=2):
            # emit `piece` ck-accumulation matmuls of the current wave-2
            # group; returns True while work remains
            if w2_state["tile"] is None:
                if not w2_jobs:
                    return False
                s, tch = w2_jobs[0]
                w2_state["tile"] = ps_w2_pool.tile(
                    [128, QCH], F32, tag="psv0", name=f"w2_{s}_{tch}")
                w2_state["ck"] = 0
            s, tch = w2_jobs[0]
            p = w2_state["tile"]
            for ck in range(w2_state["ck"],
                            min(8, w2_state["ck"] + piece)):
                nc.tensor.matmul(
                    p,
                    lhsT=wqk_sb[:, ck, s * 128:(s + 1) * 128],
                    rhs=xt_sb[:, ck, tch * QCH:(tch + 1) * QCH],
                    start=(ck == 0), stop=(ck == 7),
                )
            w2_state["ck"] += piece
            if w2_state["ck"] >= 8:
                hp, qk = SLICE_MAP[s]
                nc.vector.tensor_scalar_add(
                    out=qk_sb[:, hp, qk, tch * QCH:(tch + 1) * QCH],
                    in0=p, scalar1=bqk_sb[:, s:s + 1],
                )
                w2_jobs.pop(0)
                w2_state["tile"] = None
            return bool(w2_jobs)

    for h in range(HL):
        hp, off = h // 2, 64 * (h % 2)
        for ja, jb in PASSES:
            pv = {j: pv_pool.tile([128, QCH], F32, tag=pv_tag(j),
                                  name=f"pv{j}")
                  for j in range(ja, jb)}
            for i in range(4 * (jb - 1) + 4):  # i <= 4*(jb-1)+3
                if KVAR == "A" and h == 0 and ja == 0 and i < 8:
                    emit_v(2 * i, psv_pool, 2)
                    emit_v(2 * i + 1, psv_pool, 2)
                if KVAR == "D" and h == 0 and (
                        (ja == 0 and i < 8) or (ja == 2 and 8 <= i < 16)):
                    emit_v(i, ps_w2_pool, 1)
                elif KVAR in ("C", "D") and h < 3 and w2_jobs:
                    emit_w2(2)
                j0 = i // 4
                lo = i * 128 - j0 * QCH        # diag offset inside chunk j0
                lom = min(lo, 256)             # f32r needs >=256 moving cols
                jlist = [j for j in range(max(ja, j0), jb)]
                for ga in range(0, len(jlist), 2):
                    grp = jlist[ga:ga + 2]
                    ps_s = pss_pool.tile([128, 2, QCH], F32, tag="pss",
                                         name="pss")
                    st = st_pool.tile([128, 2, QCH], MMDT, tag="st", name="st")
                    for b, j in enumerate(grp):
                        lj = lom if j == j0 else 0
                        nc.tensor.matmul(
                            ps_s[:, b, lj:],
                            lhsT=qk_sb[off:off + 64, hp, 1,
                                       i * 128:(i + 1) * 128],
                            rhs=qk_sb[off:off + 64, hp, 0,
                                      j * QCH + lj:(j + 1) * QCH],
                            start=True, stop=True,
                        )
                    lg = lom if grp[0] == j0 else 0
                    nc.scalar.activation(
                        out=st.rearrange("p a b -> p (a b)")[
                            :, lg:len(grp) * QCH],
                        in_=ps_s.rearrange("p a b -> p (a b)")[
                            :, lg:len(grp) * QCH],
                        func=ACT.Exp, scale=0.125,
                    )
                    if grp[0] == j0:
                        # causal mask: zero [lom:lo) + triangular diag block
                        mask = trix_sb if lo > lom else tri_sb
                        nc.vector.tensor_mul(
                            out=st[:, 0, lom:lo + 128],
                            in0=st[:, 0, lom:lo + 128], in1=mask,
                        )
                    for b, j in enumerate(grp):
                        lj = lom if j == j0 else 0
                        nc.tensor.matmul(
                            pv[j][:, lj:],
                            lhsT=v_sb[:, i, h, :],
                            rhs=st[:, b, lj:],
                            start=(i == 0), stop=(i == 4 * j + 3),
                        )
                for j in pv:
                    # chunk j's accumulation ends at i == 4j+3: normalize
                    # immediately so the bank frees while later chunks run
                    if i == 4 * j + 3:
                        norm(h, pv, j)
    rel = [atmp_pool, rc_pool, st_pool]
    if KVAR == "A":
        rel.append(psv_pool)
    if KVAR in ("C", "D"):
        rel.append(ps_w2_pool)
    rel += [pv_pool, pss_pool]
    for p in rel:
        p.release()
    if KVAR in ("C", "D"):
        pinw.release()
    pin.release()

    # ---------------- output projection ----------------
    with tc.tile_pool(name="ps_y", bufs=4, space="PSUM") as psy_pool, \
         tc.tile_pool(name="yo", bufs=4) as y_pool:
        for tt in range(NKT):
            for n2 in range(2):
                ps_y = psy_pool.tile([128, QCH], F32, tag="psy")
                for ct in range(2):
                    nc.tensor.matmul(
                        ps_y,
                        lhsT=at_sb[:, ct, tt * 128:(tt + 1) * 128],
                        rhs=wp_sb[:, ct, n2 * QCH:(n2 + 1) * QCH],
                        start=(ct == 0), stop=(ct == 1),
                    )
                yt = y_pool.tile([128, QCH], F32, tag="yt")
                nc.vector.tensor_copy(out=yt, in_=ps_y)
                nc.sync.dma_start(
                    out=y[tt * 128:(tt + 1) * 128, n2 * QCH:(n2 + 1) * QCH],
                    in_=yt,
                )

    if debug_dumps:
        qk_d = nc.dram_tensor("qk_d", [128, 2, 2, T], MMDT, kind="ExternalOutput").ap()
        v_d = nc.dram_tensor("v_d", [128, NKT, HL, 128], MMDT,
                             kind="ExternalOutput").ap()
        at_d = nc.dram_tensor("at_d", [128, 2, T], MMDT, kind="ExternalOutput").ap()
        nc.sync.dma_start(out=qk_d, in_=qk_sb)
        nc.sync.dma_start(out=v_d, in_=v_sb)
        nc.sync.dma_start(out=at_d, in_=at_sb)

    persist.release()


_PROGRAM = None


def build_program(debug_dumps=False):
    global _PROGRAM
    if _PROGRAM is None or debug_dumps:
        nc = bacc.Bacc("TRN2", debug=False, num_devices=N_CORES)
        with tile.TileContext(nc) as tc:
            _body(tc, debug_dumps=debug_dumps)
        nc.compile()
        if debug_dumps:
            return nc
        _PROGRAM = nc
    return _PROGRAM


def make_in_maps(x, W_attn, b_attn, W_proj):
    """Host-side shard: per-core input dict."""
    x = np.asarray(x, np.float32)
    W_attn = np.asarray(W_attn, np.float32)
    b_attn = np.asarray(b_attn, np.float32)
    W_proj = np.asarray(W_proj, np.float32)
    tri = np.triu(np.ones((128, 128), np.float32))  # tri[k, q] = k <= q
    trix = np.concatenate(
        [np.zeros((128, 128), np.float32), tri], axis=1)  # [0 | tri]
    in_maps = []
    for c in range(N_CORES):
        b, g = divmod(c, 4)
        xt = np.ascontiguousarray(x[b].T)  # [C, T]
        q0 = 256 * g
        cols = np.r_[q0:q0 + 256, C + q0:C + q0 + 256]  # q then k, heads 4g..4g+3
        wqk = np.ascontiguousarray(W_attn[:, cols])  # [C, 512] = [q01|q23|k01|k23]
        wv = np.ascontiguousarray(W_attn[:, 2 * C + q0:2 * C + q0 + 256])
        wp_l = np.ascontiguousarray(W_proj[q0:q0 + 256, :])
        bqk = np.ascontiguousarray(
            b_attn[cols].reshape(4, 128).T)  # [128, 4], col s = slice s bias
        in_maps.append({
            "xt": round_f32r(xt), "wqk": round_f32r(wqk),
            "wv": round_f32r(wv), "wp": round_f32r(wp_l),
            "bqk": bqk, "tri": tri, "trix": trix,
        })
    return in_maps


def kernel(x, W_attn, b_attn, W_proj, b_proj):
    global LAST_RESULT
    W_attn = np.asarray(W_attn, np.float32)
    W_proj = np.asarray(W_proj, np.float32)
    b_attn = np.asarray(b_attn, np.float32)
    b_proj = np.asarray(b_proj, np.float32)

    nc = build_program()
    in_maps = make_in_maps(x, W_attn, b_attn, W_proj)
    res = run_bass_kernel_spmd(nc, in_maps, core_ids=list(range(N_CORES)))
    LAST_RESULT = res
    parts = [r["y"] for r in res.results]
    yb = [parts[0] + parts[1] + parts[2] + parts[3],
          parts[4] + parts[5] + parts[6] + parts[7]]
    out = np.stack(yb, axis=0)  # [B, T, C]
    # host-folded biases: b_proj, and the v-part of b_attn (softmax rows sum to 1)
    out += (b_proj + b_attn[2 * C:] @ W_proj)[None, None, :]
    return out.astype(np.float32)
